# revision 1
# baseline (speedup 1.0000x reference)
"""Varlen causal GQA attention on 8 trn2 NeuronCores, head-parallel sharding.

Each core takes 2 of the 16 query heads plus their shared GQA KV head and
processes all sequences. Flash-attention style blocks of 128 tokens; scores
are computed transposed ([k, q] layout) so the AV matmul needs no transposes,
and V is augmented with a ones-column so the softmax denominator falls out of
the same PSUM accumulation. exp() without max-subtraction is exact here:
scores are O(5) and the reference's -10000 masking underflows to 0 in fp32.

Layout notes:
 - qt SBUF [128, N]: head h occupies partitions 64h..64h+63 (D=64 rows); kt is
   duplicated into both partition halves so each head's QK matmul sees lhsT
   and rhs at the same base partition -> PE row-group packing lets the two
   heads' QK matmuls overlap in the array.
 - Diagonal (i==j) score blocks for the whole head are packed 4-per-PSUM-bank,
   exp'd with one ACT call per bank and causal-masked with one broadcast
   tensor_tensor per bank into a persistent ptd buffer.
 - Off-diagonal key blocks are processed 2 per PSUM chunk with a single
   3D-access-pattern exp call.
"""

import sys

sys.path.insert(0, "/opt/trn_rl_repo")

import os
import numpy as np
import ml_dtypes

DEBUG_PTD = os.environ.get("DEBUG_PTD", "0") == "1"

import concourse.bass as bass
import concourse.mybir as mybir
import concourse.tile as tile
import concourse.bacc as bacc
from concourse.bass_utils import run_bass_kernel_spmd

N_CORES = 8
H = 16
HKV = 4
D = 64
HPC = H // N_CORES  # heads per core
BLK = 128
GRP = 512  # q tokens per group (4 blocks, one PSUM bank wide)
SCALE = 0.125  # 1/sqrt(64)

BF16 = mybir.dt.bfloat16
F32 = mybir.dt.float32
np_bf16 = ml_dtypes.bfloat16
Exp = mybir.ActivationFunctionType.Exp

_cache = {}


def _build(lens):
    """Build the SPMD Bass program for per-seq padded lengths (multiples of 128)."""
    lens = [int(L) for L in lens]
    Ts = [L // BLK for L in lens]
    starts = [0]
    for L in lens:
        starts.append(starts[-1] + L)
    N = starts[-1]
    NB = N // BLK

    nc = bacc.Bacc("TRN2", target_bir_lowering=False, debug=False,
                   num_devices=N_CORES)

    qT_d = nc.dram_tensor("qT", [HPC * D, N], BF16, kind="ExternalInput")
    kT_d = nc.dram_tensor("kT", [D, N], BF16, kind="ExternalInput")
    v_d = nc.dram_tensor("v", [N, D + 1], BF16, kind="ExternalInput")
    mtri_d = nc.dram_tensor("mtri", [BLK, 4 * BLK], BF16, kind="ExternalInput")
    oT_d = nc.dram_tensor("oT", [HPC, D + 1, N], F32, kind="ExternalOutput")

    with tile.TileContext(nc) as tc:
        with (
            tc.tile_pool(name="const", bufs=1) as const,
            tc.tile_pool(name="pt_pool", bufs=3) as pt_pool,
            tc.tile_pool(name="ot_pool", bufs=3) as ot_pool,
            tc.tile_pool(name="ps_o", bufs=3, space="PSUM") as ps_o,
            tc.tile_pool(name="ps_po", bufs=1, space="PSUM") as ps_po,
        ):
            qt = const.tile([HPC * D, N], BF16)
            kt = const.tile([2 * D, N], BF16)
            vp = const.tile([BLK, NB * (D + 1)], BF16)
            mtri = const.tile([BLK, 4 * BLK], BF16)
            nc.sync.dma_start(mtri[:], mtri_d[:])
            qs_step = (N // 4 // BLK) * BLK
            for c0 in range(0, N, qs_step):
                c1 = min(c0 + qs_step, N)
                nc.sync.dma_start(kt[0:D, c0:c1], kT_d[:, c0:c1])
                nc.sync.dma_start(kt[D:2 * D, c0:c1], kT_d[:, c0:c1])
                nc.sync.dma_start(qt[:, c0:c1], qT_d[:, c0:c1])
            vp_view = vp.rearrange("p (j e) -> p j e", e=D + 1)
            nc.sync.dma_start(vp_view[:], v_d.rearrange("(j p) e -> p j e", p=BLK))

            def kt_ap(jg, h):
                return kt[D * h:D * h + D, BLK * jg:BLK * jg + BLK]

            def vp_ap(jg):
                return vp[:, jg * (D + 1):(jg + 1) * (D + 1)]

            # ---- phase A: all diagonal-region wedges, batched by width ----
            # wedge (b, g, r): k-block jg = q0/128 + r, q-cols [q0+128r, q0+W)
            groups = []  # (q0, W, s0, g) in main-loop execution order
            for b, Lb in enumerate(lens):
                s0 = starts[b]
                T = Ts[b]
                for g in range((T + 3) // 4):
                    q0 = s0 + GRP * g
                    W = min(GRP, Lb - GRP * g)
                    work = 4 * g * (W // BLK) + (W // BLK) ** 2
                    groups.append((work, q0, W, s0, g))
            groups.sort(key=lambda t: -t[0])
            groups = [t[1:] for t in groups]
            wedges = []  # (jg, qs, width, gidx) in deterministic order
            for gidx, (q0, W, s0, g) in enumerate(groups):
                for r in range(W // BLK):
                    wedges.append((q0 // BLK + r, q0 + BLK * r,
                                   W - BLK * r, gidx))
            # assign ptw offsets grouped by width class
            SLOTS = {512: 2, 384: 2, 256: 4, 128: 8}
            STRIDE = {512: 512, 384: 512, 256: 256, 128: 128}
            by_w = {}
            for jg, qs, wd, gi in wedges:
                by_w.setdefault(wd, []).append((jg, qs, gi))
            ptw_off = {}
            off = 0
            raw_chunks = []  # (min_gidx, width, [(jg, qs, off)...])
            for wd in sorted(by_w, reverse=True):
                lst = by_w[wd]
                s = SLOTS[wd]
                for c0 in range(0, len(lst), s):
                    grp = []
                    need = 10 ** 9
                    for jg, qs, gi in lst[c0:c0 + s]:
                        ptw_off[jg] = off
                        grp.append((jg, qs, off))
                        off += wd
                        need = min(need, gi)
                    raw_chunks.append((need, wd, grp))
            # emit chunks in the order the main loop needs them
            chunks = [(wd, grp) for _need, wd, grp in
                      sorted(raw_chunks, key=lambda t: (t[0], -t[1]))]
            ptw_sz = off
            ptw = const.tile([BLK, HPC * ptw_sz], BF16)
            ptw_h = [ptw[:, h * ptw_sz:(h + 1) * ptw_sz] for h in range(HPC)]

            for wd, grp in chunks:
                stride = STRIDE[wd]
                for h in range(HPC):
                    psc = ps_o.tile([BLK, 1024], F32, name="psc")
                    for t, (jg, qs, _o) in enumerate(grp):
                        nc.tensor.matmul(
                            psc[:, stride * t:stride * t + wd],
                            kt_ap(jg, h),
                            qt[D * h:D * h + D, qs:qs + wd],
                            start=True, stop=True,
                        )
                    o0 = grp[0][2]
                    n = len(grp)
                    dst = ptw_h[h][:, o0:o0 + n * wd]
                    if stride == wd:
                        nc.scalar.activation(dst, psc[:, :n * wd], Exp,
                                             scale=SCALE)
                    else:
                        src3 = psc.rearrange("p (t c) -> p t c", c=stride)
                        nc.scalar.activation(
                            dst.rearrange("p (t c) -> p t c", c=wd),
                            src3[:, :n, :wd], Exp, scale=SCALE)
                    for _jg, _qs, o in grp:
                        tri = ptw_h[h][:, o:o + BLK]
                        nc.vector.tensor_tensor(tri, tri, mtri[:, :BLK],
                                                mybir.AluOpType.mult)

            # ---- main: per (seq, q-group): wedge AVs then off-diag chunks ----
            for q0, W, s0, g in groups:
                if True:
                    nblk = W // BLK
                    po = [ps_po.tile([D + 1, GRP], F32, name=f"po{h}")
                          for h in range(HPC)]
                    for h in range(HPC):
                        for r in range(nblk):
                            jg = q0 // BLK + r
                            o = ptw_off[jg]
                            wd = W - BLK * r
                            nc.tensor.matmul(
                                po[h][:, BLK * r:BLK * r + wd],
                                vp_ap(jg),
                                ptw_h[h][:, o:o + wd],
                                start=(r == 0),
                                stop=(g == 0 and r == nblk - 1),
                            )
                    for c in range(2 * g):
                        js = [2 * c, 2 * c + 1]
                        for h in range(HPC):
                            ps2 = ps_o.tile([BLK, 1024], F32, name="psc")
                            for t, j in enumerate(js):
                                jg = s0 // BLK + j
                                nc.tensor.matmul(
                                    ps2[:, 512 * t:512 * t + W],
                                    kt_ap(jg, h),
                                    qt[D * h:D * h + D, q0:q0 + W],
                                    start=True, stop=True,
                                )
                            pto = pt_pool.tile([BLK, 1024], BF16)
                            if W == GRP:
                                nc.scalar.activation(pto[:], ps2[:], Exp,
                                                     scale=SCALE)
                            else:
                                nc.scalar.activation(
                                    pto.rearrange("p (t c) -> p t c", c=512)[:, :, :W],
                                    ps2.rearrange("p (t c) -> p t c", c=512)[:, :, :W],
                                    Exp, scale=SCALE)
                            for t, j in enumerate(js):
                                jg = s0 // BLK + j
                                nc.tensor.matmul(
                                    po[h][:, :W],
                                    vp_ap(jg),
                                    pto[:, 512 * t:512 * t + W],
                                    start=False,
                                    stop=(c == 2 * g - 1 and t == 1),
                                )
                    for h in range(HPC):
                        ot = ot_pool.tile([D + 1, GRP], F32, name=f"ot{h}")
                        nc.vector.tensor_copy(ot[:, :W], po[h][:, :W])
                        nc.sync.dma_start(oT_d[h, :, q0:q0 + W], ot[:, :W])

    nc.compile()
    return nc


def kernel(q, kv, cu_seqlens, max_seqlen=None):
    q = np.asarray(q, dtype=np.float32)
    kv = np.asarray(kv, dtype=np.float32)
    cu = np.asarray(cu_seqlens).astype(np.int64)
    total = q.shape[0]
    lens_raw = (cu[1:] - cu[:-1]).tolist()
    lens = [max(BLK, -(-int(L) // BLK) * BLK) for L in lens_raw]
    key = tuple(lens)
    if key not in _cache:
        _cache[key] = _build(lens)
    nc = _cache[key]

    starts_p = np.concatenate([[0], np.cumsum(lens)])
    N = int(starts_p[-1])

    # pad ragged sequences into the 128-aligned packed layout
    qp = np.zeros((N, H, D), np.float32)
    kp = np.zeros((N, HKV, D), np.float32)
    v65 = np.zeros((N, HKV, D + 1), np.float32)
    for b in range(len(lens)):
        s, e = int(cu[b]), int(cu[b + 1])
        d0 = int(starts_p[b])
        qp[d0:d0 + e - s] = q[s:e]
        kp[d0:d0 + e - s] = kv[s:e, 0]
        v65[d0:d0 + e - s, :, :D] = kv[s:e, 1]
    v65[:, :, D] = 1.0

    # [p, x] = x >= p, tiled 4x along free dim for packed diagonal chunks
    mtri = np.tile(np.triu(np.ones((BLK, BLK), np_bf16)), (1, 4))

    in_maps = []
    for c in range(N_CORES):
        hq = [HPC * c + i for i in range(HPC)]
        ckv = hq[0] // (H // HKV)
        qT = np.ascontiguousarray(
            qp[:, hq, :].transpose(1, 2, 0).reshape(HPC * D, N)
        ).astype(np_bf16)
        kT = np.ascontiguousarray(kp[:, ckv, :].T).astype(np_bf16)  # [64, N]
        vc = np.ascontiguousarray(v65[:, ckv, :]).astype(np_bf16)
        in_maps.append({"qT": qT, "kT": kT, "v": vc, "mtri": mtri})

    res = run_bass_kernel_spmd(nc, in_maps, core_ids=list(range(N_CORES)))
    global last_results
    last_results = res

    out = np.empty((total, H, D), np.float32)
    for c in range(N_CORES):
        oT = res.results[c]["oT"]  # [HPC, 65, N]
        o = oT[:, :D, :] / oT[:, D:D + 1, :]  # [HPC, 64, N]
        o = o.transpose(2, 0, 1)  # [N, HPC, 64]
        for b in range(len(lens)):
            s, e = int(cu[b]), int(cu[b + 1])
            d0 = int(starts_p[b])
            out[s:e, HPC * c:HPC * (c + 1), :] = o[d0:d0 + e - s]
    return out



# revision 2
# speedup vs baseline: 1.3331x; 1.3331x over previous
"""Varlen causal GQA attention on 8 trn2 NeuronCores, head-parallel sharding.

Each core takes 2 of the 16 query heads plus their shared GQA KV head and
processes all sequences. v2 design, tuned against the TimelineSim cost model:

 - QK scores per (q-block, k-block) 128x128 tile in [k, q] layout, bf16.
 - AV is *transposed*: the exp'd score block is the matmul's stationary
   operand (lhsT) and V (augmented with a ones column for the softmax
   denominator) is streamed -> only 65 columns per block instead of 128.
 - exp() is split across two engines: ACT does exact exp for diagonal
   blocks (plus a load-balanced share of off-diagonal chunks); the DVE does
   the rest with a 1-instruction Schraudolph exp (f32 scores -> int16
   round-saturate cast of x*a+b == bf16 bit pattern of ~e^x, +-3% rel).
   Diagonal blocks are masked by a bf16 triangular multiply on DVE.
 - exp without max-subtraction is exact here: scores are O(5) in fp32.
 - Outputs (numerator + denominator) are copied f32->bf16 to SBUF staging
   (ACT/DVE balanced) and DMA'd per (seq, head); the host divides.
"""

import sys

sys.path.insert(0, "/opt/trn_rl_repo")

import math
import numpy as np
import ml_dtypes

import concourse.bass as bass
import concourse.mybir as mybir
import concourse.tile as tile
import concourse.bacc as bacc
from concourse.bass_utils import run_bass_kernel_spmd

N_CORES = 8
H = 16
HKV = 4
D = 64
HPC = H // N_CORES  # query heads per core
BLK = 128
SCALE = 0.125  # 1/sqrt(64)

BF16 = mybir.dt.bfloat16
I16 = mybir.dt.int16
F32 = mybir.dt.float32
np_bf16 = ml_dtypes.bfloat16
Exp = mybir.ActivationFunctionType.Exp
Copy = mybir.ActivationFunctionType.Copy
Mult = mybir.AluOpType.mult
Add = mybir.AluOpType.add

# Schraudolph: bits_bf16(e^(SCALE*x)) ~= rint(x*SCH_A + SCH_B), saturating.
SCH_CENTER = 3.66
SCH_A = SCALE * math.log2(math.e) * 128.0
SCH_B = 127.0 * 128.0 - SCH_CENTER

# engine cost estimates (ns) for load balancing, from the TimelineSim model
ACT_RATE, ACT_OV = 0.8333, 185.0
DVE_RATE, DVE_OV = 1.0417, 125.0

_cache = {}


def _chunk(pairs, maxb=8):
    """Split the (i, j) off-diag block list into balanced chunks of <=maxb."""
    n = len(pairs)
    if n == 0:
        return []
    k = -(-n // maxb)
    base, rem = divmod(n, k)
    out, p = [], 0
    for c in range(k):
        sz = base + (1 if c < rem else 0)
        out.append(pairs[p:p + sz])
        p += sz
    return out


def _build(lens):
    lens = [int(L) for L in lens]
    Ts = [L // BLK for L in lens]
    NB = sum(Ts)
    N = NB * BLK
    sblk = [0]
    for T in Ts:
        sblk.append(sblk[-1] + T)

    nc = bacc.Bacc("TRN2", target_bir_lowering=False, debug=False,
                   num_devices=N_CORES)

    qT_d = nc.dram_tensor("qT", [HPC * D, N], BF16, kind="ExternalInput")
    kT_d = nc.dram_tensor("kT", [D, N], BF16, kind="ExternalInput")
    v_d = nc.dram_tensor("v", [BLK, NB * (D + 1)], BF16, kind="ExternalInput")
    mtri_d = nc.dram_tensor("mtri", [BLK, 4 * BLK], BF16, kind="ExternalInput")
    oT_d = nc.dram_tensor("oT", [HPC, BLK, NB * (D + 1)], BF16,
                          kind="ExternalOutput")

    eng_ns = {"A": 0.0, "D": 0.0}  # running engine-load estimate

    with tile.TileContext(nc) as tc:
        with (
            tc.tile_pool(name="const", bufs=1) as const,
            tc.tile_pool(name="sd_pool", bufs=3) as sd_pool,
            tc.tile_pool(name="so_pool", bufs=6) as so_pool,
            tc.tile_pool(name="ps_pso", bufs=3, space="PSUM") as ps_pso,
            tc.tile_pool(name="ps_pd", bufs=1, space="PSUM") as ps_pd,
            tc.tile_pool(name="ps_po", bufs=1, space="PSUM") as ps_po,
        ):
            qt = const.tile([HPC * D, N], BF16)
            kt = const.tile([2 * D, N], BF16)
            vp = const.tile([BLK, NB * (D + 1)], BF16)
            mtri = const.tile([BLK, 4 * BLK], BF16)
            stg = const.tile([BLK, HPC * NB * (D + 1)], BF16)

            nc.sync.dma_start(mtri[:], mtri_d[:])
            # input loads, chunked by sequence ranges so compute starts early
            cuts = [0, 1, 4, len(lens)]
            for c0, c1 in zip(cuts[:-1], cuts[1:]):
                t0, t1 = sblk[c0] * BLK, sblk[c1] * BLK
                if t0 == t1:
                    continue
                nc.sync.dma_start(kt[0:D, t0:t1], kT_d[:, t0:t1])
                nc.sync.dma_start(kt[D:2 * D, t0:t1], kT_d[:, t0:t1])
                nc.sync.dma_start(qt[:, t0:t1], qT_d[:, t0:t1])
                b0, b1 = sblk[c0] * (D + 1), sblk[c1] * (D + 1)
                nc.sync.dma_start(vp[:, b0:b1], v_d[:, b0:b1])

            def kap(h, jg):
                return kt[D * h:D * h + D, BLK * jg:BLK * jg + BLK]

            def qap(h, ig, w=BLK):
                return qt[D * h:D * h + D, BLK * ig:BLK * ig + w]

            def vap(jg):
                return vp[:, (D + 1) * jg:(D + 1) * (jg + 1)]

            units = []
            for b, T in enumerate(Ts):
                for g in range(-(-T // 4)):
                    I = list(range(4 * g, min(4 * g + 4, T)))
                    units.append((b, g, I))

            def pick(cost_a, cost_d):
                if eng_ns["A"] + cost_a <= eng_ns["D"] + cost_d:
                    eng_ns["A"] += cost_a
                    return "A"
                eng_ns["D"] += cost_d
                return "D"

            def emit_front(b, g, I, h):
                """QK + exp for one unit; returns AV bookkeeping."""
                s = sblk[b]
                blockmap = {}
                # off-diagonal chunks
                pairs = [(i, j) for i in I for j in range(i)]
                for ch in _chunk(pairs):
                    C = BLK * len(ch)
                    pso = ps_pso.tile([BLK, 1024], F32, name="pso")
                    for t, (i, j) in enumerate(ch):
                        nc.tensor.matmul(pso[:, BLK * t:BLK * t + BLK],
                                         kap(h, s + j), qap(h, s + i),
                                         start=True, stop=True)
                    so = so_pool.tile([BLK, 1024], BF16, name="so")
                    e = pick(C * ACT_RATE + ACT_OV, C * DVE_RATE + DVE_OV)
                    if e == "A":
                        nc.scalar.activation(so[:, :C], pso[:, :C], Exp,
                                             scale=SCALE)
                    else:
                        nc.vector.tensor_scalar(
                            so[:, :C].bitcast(I16), pso[:, :C],
                            SCH_A, SCH_B, Mult, Add)
                    for t, (i, j) in enumerate(ch):
                        blockmap[(i, j)] = (so, BLK * t)
                # diagonal chunk: exact ACT exp + DVE triangular mask
                L4 = BLK * len(I)
                pd = ps_pd.tile([BLK, 512], F32, name="pd")
                for idx, i in enumerate(I):
                    nc.tensor.matmul(pd[:, BLK * idx:BLK * idx + BLK],
                                     kap(h, s + i), qap(h, s + i),
                                     start=True, stop=True)
                sd = sd_pool.tile([BLK, 512], BF16, name="sd")
                nc.scalar.activation(sd[:, :L4], pd[:, :L4], Exp, scale=SCALE)
                eng_ns["A"] += L4 * ACT_RATE + ACT_OV
                nc.vector.tensor_tensor(sd[:, :L4], sd[:, :L4], mtri[:, :L4],
                                        Mult)
                eng_ns["D"] += L4 * 0.52 + 60.0
                for idx, i in enumerate(I):
                    blockmap[(i, i)] = (sd, BLK * idx)
                return (b, g, I, h, blockmap)

            n_done = {}  # (b, h) -> units completed, for output DMA flush

            def emit_back(u):
                b, g, I, h, blockmap = u
                s = sblk[b]
                po = ps_po.tile([BLK, 512], F32, name="po")
                for idx, i in enumerate(I):
                    for j in range(i + 1):
                        t, c = blockmap[(i, j)]
                        nc.tensor.matmul(
                            po[:, (D + 1) * idx:(D + 1) * idx + (D + 1)],
                            t[:, c:c + BLK], vap(s + j),
                            start=(j == 0), stop=(j == i))
                CW = (D + 1) * len(I)
                o0 = (h * NB + s + 4 * g) * (D + 1)
                dst = stg[:, o0:o0 + CW]
                e = pick(CW * ACT_RATE + ACT_OV, CW * DVE_RATE + DVE_OV)
                if e == "A":
                    nc.scalar.activation(dst, po[:, :CW], Copy)
                else:
                    nc.vector.tensor_copy(dst, po[:, :CW])
                # flush this (seq, head) when its last group is copied
                n_done[(b, h)] = n_done.get((b, h), 0) + 1
                if n_done[(b, h)] == -(-Ts[b] // 4):
                    c0 = (h * NB + sblk[b]) * (D + 1)
                    cw = Ts[b] * (D + 1)
                    nc.sync.dma_start(
                        oT_d[h, :, sblk[b] * (D + 1):(sblk[b] + Ts[b]) * (D + 1)],
                        stg[:, c0:c0 + cw])

            prev = None
            for b, g, I in units:
                for h in range(HPC):
                    u = emit_front(b, g, I, h)
                    if prev is not None:
                        emit_back(prev)
                    prev = u
            emit_back(prev)

    nc.compile()
    return nc


def kernel(q, kv, cu_seqlens, max_seqlen=None):
    q = np.asarray(q, dtype=np.float32)
    kv = np.asarray(kv, dtype=np.float32)
    cu = np.asarray(cu_seqlens).astype(np.int64)
    total = q.shape[0]
    lens_raw = (cu[1:] - cu[:-1]).tolist()
    lens = [max(BLK, -(-int(L) // BLK) * BLK) for L in lens_raw]
    key = tuple(lens)
    if key not in _cache:
        _cache[key] = _build(lens)
    nc = _cache[key]

    starts_p = np.concatenate([[0], np.cumsum(lens)])
    N = int(starts_p[-1])
    NB = N // BLK

    qp = np.zeros((N, H, D), np.float32)
    kp = np.zeros((N, HKV, D), np.float32)
    v65 = np.zeros((N, HKV, D + 1), np.float32)
    for b in range(len(lens)):
        s, e = int(cu[b]), int(cu[b + 1])
        d0 = int(starts_p[b])
        qp[d0:d0 + e - s] = q[s:e]
        kp[d0:d0 + e - s] = kv[s:e, 0]
        v65[d0:d0 + e - s, :, :D] = kv[s:e, 1]
        v65[d0:d0 + e - s, :, D] = 1.0  # ones only on valid tokens

    mtri = np.tile(np.triu(np.ones((BLK, BLK), np_bf16)), (1, 4))

    in_maps = []
    for c in range(N_CORES):
        hq = [HPC * c + i for i in range(HPC)]
        ckv = hq[0] // (H // HKV)
        qT = np.ascontiguousarray(
            qp[:, hq, :].transpose(1, 2, 0).reshape(HPC * D, N)
        ).astype(np_bf16)
        kT = np.ascontiguousarray(kp[:, ckv, :].T).astype(np_bf16)
        vc = np.ascontiguousarray(
            v65[:, ckv, :].reshape(NB, BLK, D + 1).transpose(1, 0, 2)
            .reshape(BLK, NB * (D + 1))).astype(np_bf16)
        in_maps.append({"qT": qT, "kT": kT, "v": vc, "mtri": mtri})

    res = run_bass_kernel_spmd(nc, in_maps, core_ids=list(range(N_CORES)))
    global last_results
    last_results = res

    out = np.empty((total, H, D), np.float32)
    for c in range(N_CORES):
        oT = np.asarray(res.results[c]["oT"]).astype(np.float32)
        # [HPC, 128, NB*65] -> per head [NB,128,65] token-major
        o = oT.reshape(HPC, BLK, NB, D + 1).transpose(0, 2, 1, 3)
        o = o.reshape(HPC, N, D + 1)
        num = o[:, :, :D]
        den = o[:, :, D:D + 1]
        val = num / den
        for b in range(len(lens)):
            s, e = int(cu[b]), int(cu[b + 1])
            d0 = int(starts_p[b])
            out[s:e, HPC * c:HPC * (c + 1), :] = val[:, d0:d0 + e - s].transpose(1, 0, 2)
    return out
